# revision 1
# baseline (speedup 1.0000x reference)
"""PWC-Net correlation (nn_CorrBlock) Trainium2 Bass kernel — 2D-tile Gram.

Problem: feat1, feat2 [8, 256, 80, 160] f32 -> leaky_relu(corr, 0.1)
  corr[n, d, h, w] = (1/256) * sum_c feat1[n,c,h,w] * feat2p[n,c,h+dy,w+dx]
  d = 9*dy + dx, (dy, dx) in [0..8]^2, feat2p zero-padded by 4.

Strategy (data-parallel, 1 sample per NeuronCore, 8 cores):
  - Cast-load both feature maps to SBUF as bf16 (feat2 zero-padded to
    [88 x 168]), channels on partitions (2 chunks of 128); feat1 is
    pre-scaled in place by 1/256 (exact exponent shift in bf16).
  - 2D pixel tiles of TH x TW = 8 x 16 = 128 pixels. Per tile, the taps
    any pixel needs form one 16 x 24 = 384-tap patch of feat2p. Two
    matmuls (C chunks) compute the full cross product
    T[pixel, tap] = sum_c f1[c, pixel] * f2p[c, tap] in PSUM [128, 384]:
    6 streamed columns per pixel vs ~20 for a row-Gram.
  - Drain+leaky-relu PSUM->SBUF bf16, split ACT (Prelu activation,
    first SA cols) / DVE (scalar_tensor_tensor mult+max, rest).
  - One batched DMA per tile row stores the 10 drained patches
    [128, 3840] bf16 to DRAM (contiguous 768B+ runs).
  - Host extracts each pixel's 9x9 window from its patch (pure gather)
    and casts to f32.
"""

import sys

sys.path.insert(0, "/opt/trn_rl_repo")
import numpy as np

N, C, H, W = 8, 256, 80, 160
HP, WP = 88, 168  # padded feat2 dims (+4 each side)
TH, TW = 8, 16  # pixel tile
PH, PW = TH + 8, TW + 8  # tap patch per tile
PATCH = PH * PW  # 384
BANK = 512  # PSUM bank stride (f32) for bank-aligned matmul outputs
SROW = 9 * PW  # stored patch rows per pixel: only di..di+8 of the 16
NTI, NTJ = H // TH, W // TW  # 10 x 10 tiles
SA = 280  # ACT drain columns (Prelu); DVE copy-drains PATCH-SA + relus them
G, NG = 10, 8  # h-group size for load interleaving
SCALE = 1.0 / C

_cache = {}


def _build(repeat=1, use_prelu=True):
    import concourse.tile as tile
    from concourse import bacc, mybir
    from concourse.ap import AP

    F32, BF16 = mybir.dt.float32, mybir.dt.bfloat16
    nc = bacc.Bacc("TRN2", target_bir_lowering=False, debug=False)
    f1 = nc.dram_tensor("f1", [C, H * W], F32, kind="ExternalInput")
    f2 = nc.dram_tensor("f2", [C, H * W], F32, kind="ExternalInput")
    O = nc.dram_tensor("O", [NTI * NTJ * 128, PATCH], BF16, kind="ExternalOutput")

    with tile.TileContext(nc) as tc:
        with (
            tc.tile_pool(name="inp", bufs=1) as inp,
            tc.tile_pool(name="work", bufs=2) as work,
            tc.tile_pool(name="ps", bufs=2, space="PSUM") as ps,
        ):
            f1sb, f2sb = [], []
            for cc in range(2):
                t1 = inp.tile([128, H * W], BF16, tag=f"f1_{cc}")
                f1sb.append(t1)
                t2 = inp.tile([128, HP * WP], BF16, tag=f"f2_{cc}")
                a = t2[:]
                pp = a.ap[0][0]
                # zero pads: top 4 rows, bottom 4 rows, left pad of row 4,
                # then fused right(h)+left(h+1) pads of the 80 data rows
                nc.vector.memset(t2[:, 0 : 4 * WP], 0.0)
                nc.vector.memset(t2[:, 84 * WP : 88 * WP], 0.0)
                nc.vector.memset(t2[:, 4 * WP : 4 * WP + 4], 0.0)
                lr = AP(a.tensor, a.offset + 4 * WP + 164, [[pp, 128], [WP, 80], [1, 8]])
                nc.vector.memset(lr, 0.0)
                f2sb.append(t2)
            # loads emitted interleaved in consumer (h-group) order so the
            # first tile rows' working set arrives before later pieces
            for g in range(NG + 1):
                for cc in range(2):
                    a = f2sb[cc][:]
                    pp = a.ap[0][0]
                    src2 = f2.ap()[128 * cc : 128 * (cc + 1), :].rearrange(
                        "c (h w) -> c h w", h=H
                    )
                    hp_lo, hp_hi = 10 * g, min(10 * g + 10, HP)
                    d_lo, d_hi = max(hp_lo, 4), min(hp_hi, 84)
                    if d_lo < d_hi:
                        dst = AP(
                            a.tensor,
                            a.offset + d_lo * WP + 4,
                            [[pp, 128], [WP, d_hi - d_lo], [1, W]],
                        )
                        nc.gpsimd.dma_start(dst, src2[:, d_lo - 4 : d_hi - 4, :])
                if g < NG:
                    for cc in range(2):
                        fsrc = f1.ap()[128 * cc : 128 * (cc + 1), :]
                        sl = f1sb[cc][:][:, g * G * W : (g + 1) * G * W]
                        nc.gpsimd.dma_start(sl, fsrc[:, g * G * W : (g + 1) * G * W])
                        # pre-scale by 1/256 in place (exact in bf16)
                        nc.vector.tensor_scalar_mul(sl, sl, SCALE)

            for _rep in range(repeat):
                for ti in range(NTI):
                    V = work.tile([128, NTJ * PATCH], BF16, tag="V")
                    v = V[:]
                    vp = v.ap[0][0]
                    h0 = ti * TH
                    # process tj in groups of 4 sharing one 4-bank PSUM tile;
                    # matmul outputs are bank-aligned (512 f32 apart) so one
                    # grouped drain op covers the group with a 3D AP
                    for tj0 in range(0, NTJ, 4):
                        ng = min(4, NTJ - tj0)
                        T4 = ps.tile([128, 4 * BANK], F32, tag="T4")
                        t4 = T4[:]
                        tp = t4.ap[0][0]
                        for k in range(ng):
                            tj = tj0 + k
                            w0 = tj * TW
                            for cc in range(2):
                                # f1 host-tiled: tile's 128 pixels contiguous
                                a1 = f1sb[cc][:]
                                p1 = a1.ap[0][0]
                                lhsT = AP(
                                    a1.tensor,
                                    a1.offset + (ti * NTJ + tj) * 128,
                                    [[p1, 128], [1, 128]],
                                )
                                a2 = f2sb[cc][:]
                                p2 = a2.ap[0][0]
                                rhs = AP(
                                    a2.tensor,
                                    a2.offset + h0 * WP + w0,
                                    [[p2, 128], [WP, PH], [1, PW]],
                                )
                                nc.tensor.matmul(
                                    T4[:, k * BANK : k * BANK + PATCH],
                                    lhsT,
                                    rhs,
                                    start=(cc == 0),
                                    stop=(cc == 1),
                                )
                        # grouped drains: ACT Prelu on cols [0,SA), DVE
                        # copy-drain [SA,PATCH) then 4x-mode bf16 relu
                        asrc = AP(t4.tensor, t4.offset, [[tp, 128], [BANK, ng], [1, SA]])
                        adst = AP(
                            v.tensor,
                            v.offset + tj0 * PATCH,
                            [[vp, 128], [PATCH, ng], [1, SA]],
                        )
                        if use_prelu:
                            nc.scalar.activation(
                                adst,
                                asrc,
                                mybir.ActivationFunctionType.Prelu,
                                bias=0.0,
                                scale=1.0,
                                alpha=0.1,
                            )
                        else:
                            nc.scalar.mul(adst, asrc, 1.0)
                        dsrc = AP(
                            t4.tensor, t4.offset + SA, [[tp, 128], [BANK, ng], [1, PATCH - SA]]
                        )
                        ddst = AP(
                            v.tensor,
                            v.offset + tj0 * PATCH + SA,
                            [[vp, 128], [PATCH, ng], [1, PATCH - SA]],
                        )
                        nc.vector.tensor_copy(ddst, dsrc)
                        nc.vector.scalar_tensor_tensor(
                            ddst,
                            ddst,
                            0.1,
                            ddst,
                            op0=mybir.AluOpType.mult,
                            op1=mybir.AluOpType.max,
                        )
                        if not use_prelu:
                            nc.vector.scalar_tensor_tensor(
                                adst,
                                adst,
                                0.1,
                                adst,
                                op0=mybir.AluOpType.mult,
                                op1=mybir.AluOpType.max,
                            )
                    # batched store: 10 patches -> O[(ti*NTJ+tj)*128 + p, :]
                    osrc = AP(v.tensor, v.offset, [[vp, 128], [PATCH, NTJ], [1, PATCH]])
                    odst = AP(
                        O.ap().tensor,
                        ti * NTJ * 128 * PATCH,
                        [[PATCH, 128], [128 * PATCH, NTJ], [1, PATCH]],
                    )
                    nc.sync.dma_start(odst, osrc)

    nc.compile()
    return nc


def _get_nc(repeat=1, use_prelu=True):
    key = ("nc", repeat, use_prelu)
    if key not in _cache:
        _cache[key] = _build(repeat, use_prelu)
    return _cache[key]


def _prep_f1(f1_sample):
    """[C, H, W] -> tile-major [C, NTI*NTJ*128] so each tile's 128 pixels
    are contiguous (matmul weights need a single free dimension)."""
    t = f1_sample.reshape(C, NTI, TH, NTJ, TW).transpose(0, 1, 3, 2, 4)
    return np.ascontiguousarray(t.reshape(C, H * W), dtype=np.float32)


# host-side window gather indices: value = P[.., di, dj, di+dy, dj+dx]
_DI = np.arange(TH)[:, None, None, None]
_DJ = np.arange(TW)[None, :, None, None]
_DY = np.arange(9)[None, None, :, None]
_DX = np.arange(9)[None, None, None, :]


def _unpack(out_raw):
    """[n, NTI*NTJ*128, PATCH] patches -> [n, 81, H, W] f32."""
    n = out_raw.shape[0]
    P = np.asarray(out_raw, dtype=np.float32).reshape(n, NTI, NTJ, TH, TW, PH, PW)
    Gt = P[:, :, :, _DI, _DJ, _DI + _DY, _DJ + _DX]  # [n, NTI, NTJ, TH, TW, 9, 9]
    out = Gt.transpose(0, 5, 6, 1, 3, 2, 4).reshape(n, 81, H, W)
    return np.ascontiguousarray(out)


def _run(feat1, feat2, trace=False):
    from concourse.bass_utils import run_bass_kernel_spmd

    nc = _get_nc()
    in_maps = [
        {
            "f1": _prep_f1(feat1[i]),
            "f2": np.ascontiguousarray(feat2[i].reshape(C, H * W), dtype=np.float32),
        }
        for i in range(N)
    ]
    res = run_bass_kernel_spmd(nc, in_maps, core_ids=list(range(N)), trace=trace)
    out_raw = np.stack([np.asarray(res.results[i]["O"]) for i in range(N)])
    return _unpack(out_raw), res


def kernel(feat1, feat2):
    out, _ = _run(np.asarray(feat1), np.asarray(feat2))
    return out

